# revision 1
# baseline (speedup 1.0000x reference)
"""Distributed 3-layer GraphConv GNN on 8 Trainium2 NeuronCores.

Sharding: nodes (and their incoming edges) are partitioned contiguously
across the 8 cores (2560 nodes / 20 blocks of 128 dst nodes per core).
Per layer, each core:
  - dma_gathers the source-node feature rows for its edges (sorted by dst,
    padded to a uniform chunk count so the SPMD program is identical on
    every core),
  - segment-sums them per 128-dst block on the TensorEngine via one-hot
    matmuls (one-hot built on the VectorEngine with is_equal against an
    iota constant),
  - applies the dense W_rel/W_root transform + bias + ReLU,
  - transposes to node-major and AllGathers the full feature matrix so the
    next layer can gather from it.
Graph pooling is a segment-sum over the (sorted) batch vector done locally
with the same one-hot matmul trick, AllReduced across cores, followed by
the output projection and log_softmax (all computed redundantly per core).
"""
import sys

sys.path.insert(0, "/opt/trn_rl_repo")

from contextlib import ExitStack

import numpy as np

import concourse.bass as bass
import concourse.tile as tile
from concourse import bacc, mybir
from concourse.bass_utils import run_bass_kernel_spmd
from concourse.library_config import mlp as mlp_lib

N, E, F_IN, H, C_OUT, G = 20000, 640000, 64, 128, 10, 128
NCORES = 8
NPC = 2560          # nodes per core
NBLK = NPC // 128   # dst blocks per core (20)
NPAD = NCORES * NPC  # 20480
F32 = mybir.dt.float32
AF = mybir.ActivationFunctionType
ALU = mybir.AluOpType


def _prep_inputs(x, edge_index, batch):
    """Host-side edge partitioning/padding. Returns per-core input dicts
    (minus weights) and the uniform chunks-per-block count."""
    src = np.asarray(edge_index[0], dtype=np.int64)
    dst = np.asarray(edge_index[1], dtype=np.int64)
    batch = np.asarray(batch, dtype=np.int64)
    x = np.ascontiguousarray(np.asarray(x, dtype=np.float32))

    order = np.argsort(dst, kind="stable")
    dst_s = dst[order]
    src_s = src[order]
    nblk_glob = NCORES * NBLK  # 160 (157 contain real nodes)
    starts = np.searchsorted(dst_s, np.arange(nblk_glob) * 128)
    ends = np.searchsorted(dst_s, (np.arange(nblk_glob) + 1) * 128)
    counts = ends - starts
    cchunks = max(1, int(np.ceil(counts.max() / 128)))
    L = cchunks * 128  # padded edges per block

    iota = np.tile(np.arange(128, dtype=np.float32), (128, 1))
    ident = np.eye(128, dtype=np.float32)
    ones_row = np.ones((1, 128), np.float32)

    in_maps = []
    for k in range(NCORES):
        src_pad = np.zeros((NBLK, L), np.int64)
        dstrel_pad = np.full((NBLK, L), -1.0, np.float32)
        for b in range(NBLK):
            gb = k * NBLK + b
            s, e = starts[gb], ends[gb]
            n = e - s
            if n:
                src_pad[b, :n] = src_s[s:e]
                dstrel_pad[b, :n] = (dst_s[s:e] - gb * 128).astype(np.float32)
        # dma_gather idx layout: idx i at [i % 16, i // 16], replicated
        # across the 8 groups of 16 partitions.
        idx16 = src_pad.reshape(NBLK, L // 16, 16).transpose(0, 2, 1)
        idx_t = np.concatenate(list(np.tile(idx16, (1, 8, 1))), axis=1)
        idx_t = idx_t.astype(np.int16)
        # dst_rel layout: edge e = c*128 + p at [p, c]
        dr = dstrel_pad.reshape(NBLK, cchunks, 128).transpose(0, 2, 1)
        dr_t = np.ascontiguousarray(np.concatenate(list(dr), axis=1))

        # graph id per local node, [128, NBLK]; -1 for pad nodes
        gids = np.full((NBLK, 128), -1.0, np.float32)
        base = k * NPC
        valid = max(0, min(NPC, N - base))
        if valid:
            flat = np.full(NPC, -1.0, np.float32)
            flat[:valid] = batch[base : base + valid].astype(np.float32)
            gids = flat.reshape(NBLK, 128)
        batchrel_t = np.ascontiguousarray(gids.T)  # [128, NBLK]

        xT = np.zeros((F_IN, NPC), np.float32)
        if valid:
            xT[:, :valid] = x[base : base + valid].T

        in_maps.append(
            {
                "x_full": x,
                "idx_t": idx_t,
                "dstrel_t": dr_t,
                "batchrel_t": batchrel_t,
                "xT_t": np.ascontiguousarray(xT),
                "iota_t": iota,
                "ident_t": ident,
                "ones_t": ones_row,
            }
        )
    return in_maps, cchunks


def _build_program(cchunks, active_blocks=NBLK, active_chunks=None):
    """active_blocks/active_chunks < full sizes build a truncated program
    (for bisection/debug only — output is numerically wrong)."""
    L = cchunks * 128
    a_blk = active_blocks
    a_chk = active_chunks or cchunks
    nc = bacc.Bacc("TRN2", target_bir_lowering=False, debug=False,
                   num_devices=NCORES)

    x_full = nc.dram_tensor("x_full", [N, F_IN], F32, kind="ExternalInput")
    idx_t = nc.dram_tensor("idx_t", [128, NBLK * L // 16], mybir.dt.int16,
                           kind="ExternalInput")
    dstrel_t = nc.dram_tensor("dstrel_t", [128, NBLK * cchunks], F32,
                              kind="ExternalInput")
    batchrel_t = nc.dram_tensor("batchrel_t", [128, NBLK], F32,
                                kind="ExternalInput")
    xT_t = nc.dram_tensor("xT_t", [F_IN, NPC], F32, kind="ExternalInput")
    iota_t = nc.dram_tensor("iota_t", [128, 128], F32, kind="ExternalInput")
    ident_t = nc.dram_tensor("ident_t", [128, 128], F32, kind="ExternalInput")
    ones_t = nc.dram_tensor("ones_t", [1, 128], F32, kind="ExternalInput")
    w_rel_in = [nc.dram_tensor(f"w{i}_rel", [F_IN if i == 1 else H, H], F32,
                               kind="ExternalInput") for i in (1, 2, 3)]
    w_root_in = [nc.dram_tensor(f"w{i}_root", [F_IN if i == 1 else H, H], F32,
                                kind="ExternalInput") for i in (1, 2, 3)]
    b_in = [nc.dram_tensor(f"b{i}", [H, 1], F32, kind="ExternalInput")
            for i in (1, 2, 3)]
    w_out_in = nc.dram_tensor("w_out", [H, C_OUT], F32, kind="ExternalInput")
    b_out_in = nc.dram_tensor("b_out", [1, C_OUT], F32, kind="ExternalInput")
    out_t = nc.dram_tensor("out", [G, C_OUT], F32, kind="ExternalOutput")

    with tile.TileContext(nc) as tc, ExitStack() as ctx:
        const = ctx.enter_context(tc.tile_pool(name="const", bufs=1))
        feat = ctx.enter_context(tc.tile_pool(name="feat", bufs=1))
        xe_pool = ctx.enter_context(tc.tile_pool(name="xe", bufs=3))
        m_pool = ctx.enter_context(tc.tile_pool(name="m", bufs=4))
        nm_pool = ctx.enter_context(tc.tile_pool(name="nm", bufs=3))
        sm_pool = ctx.enter_context(tc.tile_pool(name="sm", bufs=1))
        psA = ctx.enter_context(tc.tile_pool(name="psA", bufs=2, space="PSUM"))
        psB = ctx.enter_context(tc.tile_pool(name="psB", bufs=2, space="PSUM"))
        psT = ctx.enter_context(tc.tile_pool(name="psT", bufs=2, space="PSUM"))
        psP = ctx.enter_context(tc.tile_pool(name="psP", bufs=1, space="PSUM"))
        dram = ctx.enter_context(tc.tile_pool(name="dram", bufs=1, space="DRAM"))

        nc.gpsimd.load_library(mlp_lib)

        def load_const(name, dram_h, shape, dtype=F32):
            t = const.tile(shape, dtype, name=name)
            nc.sync.dma_start(t[:], dram_h[:].ap() if hasattr(dram_h, "ap") else dram_h[:])
            return t

        idx_sb = const.tile([128, NBLK * L // 16], mybir.dt.int16)
        nc.sync.dma_start(idx_sb[:], idx_t[:])
        dstrel_sb = const.tile([128, NBLK * cchunks], F32)
        nc.sync.dma_start(dstrel_sb[:], dstrel_t[:])
        batchrel_sb = const.tile([128, NBLK], F32)
        nc.sync.dma_start(batchrel_sb[:], batchrel_t[:])
        iota_sb = const.tile([128, 128], F32)
        nc.sync.dma_start(iota_sb[:], iota_t[:])
        ident_sb = const.tile([128, 128], F32)
        nc.sync.dma_start(ident_sb[:], ident_t[:])
        ones_sb = const.tile([1, 128], F32)
        nc.sync.dma_start(ones_sb[:], ones_t[:])
        w_rel_sb, w_root_sb, b_sb = [], [], []
        for i in range(3):
            fi = F_IN if i == 0 else H
            wr = const.tile([fi, H], F32, name=f"wrel{i}")
            nc.sync.dma_start(wr[:], w_rel_in[i][:])
            w_rel_sb.append(wr)
            wo = const.tile([fi, H], F32, name=f"wroot{i}")
            nc.sync.dma_start(wo[:], w_root_in[i][:])
            w_root_sb.append(wo)
            bb = const.tile([H, 1], F32, name=f"b{i}")
            nc.sync.dma_start(bb[:], b_in[i][:])
            b_sb.append(bb)
        wout_sb = const.tile([H, C_OUT], F32)
        nc.sync.dma_start(wout_sb[:], w_out_in[:])
        bout_sb = const.tile([1, C_OUT], F32)
        nc.sync.dma_start(bout_sb[:], b_out_in[:])

        xT_sb = feat.tile([F_IN, NPC], F32)
        nc.sync.dma_start(xT_sb[:], xT_t[:])
        h1T_sb = feat.tile([H, NPC], F32)
        h2T_sb = feat.tile([H, NPC], F32)
        h3T_sb = feat.tile([H, NPC], F32)
        aggT_sb = feat.tile([H, NPC], F32)

        h1_loc = dram.tile([NPC, H], F32)
        h2_loc = dram.tile([NPC, H], F32)
        h1_full = dram.tile([NPAD, H], F32)
        h2_full = dram.tile([NPAD, H], F32)
        pool_in = dram.tile([H, G], F32)
        pool_out = dram.tile([H, G], F32)

        def gcn_layer(li, f_in, gather_src, inT_sb, outT_sb, h_loc, h_full):
            wrel, wroot, bb = w_rel_sb[li], w_root_sb[li], b_sb[li]
            for b in range(a_blk):
                xe = xe_pool.tile([128, cchunks * 128], F32, tag="xe",
                                  name=f"xe{li}_{b}")
                GCH = 8  # chunks per dma_gather call (1024 idxs max: larger
                         # single calls crash the device)
                for g0 in range(0, a_chk, GCH):
                    g1 = min(g0 + GCH, a_chk)
                    nsub = (g1 - g0) * 128
                    xe3 = xe[:, g0 * f_in : g1 * f_in].rearrange(
                        "p (c f) -> p c f", f=f_in)
                    nc.gpsimd.dma_gather(
                        xe3, gather_src[:],
                        idx_sb[:, b * (L // 16) + g0 * 8
                               : b * (L // 16) + g0 * 8 + nsub // 16],
                        nsub, nsub, f_in)
                agg_ps = psA.tile([128, 128], F32, tag="agg",
                                  name=f"agg{li}_{b}")
                for c in range(a_chk):
                    m = m_pool.tile([128, 128], F32, tag="m",
                                    name=f"m{li}_{b}_{c}")
                    nc.vector.tensor_scalar(
                        m[:], iota_sb[:],
                        dstrel_sb[:, b * cchunks + c : b * cchunks + c + 1],
                        None, ALU.is_equal)
                    nc.tensor.matmul(
                        agg_ps[:f_in, :],
                        xe[:, c * f_in : (c + 1) * f_in],
                        m[:],
                        start=(c == 0), stop=(c == a_chk - 1))
                nc.vector.tensor_copy(
                    aggT_sb[:f_in, b * 128 : (b + 1) * 128], agg_ps[:f_in, :])
            # dense transform + bias + relu (feature-major)
            for g in range(NPC // 512):
                hp = psB.tile([H, 512], F32, tag="hp", name=f"hp{li}_{g}")
                nc.tensor.matmul(hp[:], wrel[:],
                                 aggT_sb[:f_in, g * 512 : (g + 1) * 512],
                                 start=True, stop=False)
                nc.tensor.matmul(hp[:], wroot[:],
                                 inT_sb[:f_in, g * 512 : (g + 1) * 512],
                                 start=False, stop=True)
                nc.scalar.activation(outT_sb[:, g * 512 : (g + 1) * 512],
                                     hp[:], AF.Relu, bias=bb[:])
            # node-major store + allgather for next layer's gather source
            if h_loc is not None:
                for b in range(a_blk):
                    tp = psT.tile([128, 128], F32, tag="tp",
                                  name=f"tp{li}_{b}")
                    nc.tensor.transpose(
                        tp[:], outT_sb[:, b * 128 : (b + 1) * 128],
                        ident_sb[:])
                    nm = nm_pool.tile([128, 128], F32, tag="nm",
                                      name=f"nm{li}_{b}")
                    nc.scalar.copy(nm[:], tp[:])
                    nc.sync.dma_start(h_loc[b * 128 : (b + 1) * 128, :],
                                      nm[:])
                nc.gpsimd.collective_compute(
                    "AllGather", ALU.bypass,
                    replica_groups=[list(range(NCORES))],
                    ins=[h_loc.opt()], outs=[h_full.opt()])

        gcn_layer(0, F_IN, x_full, xT_sb, h1T_sb, h1_loc, h1_full)
        gcn_layer(1, H, h1_full, h1T_sb, h2T_sb, h2_loc, h2_full)
        gcn_layer(2, H, h2_full, h2T_sb, h3T_sb, None, None)

        # ---- pooling: pooledT[h, g] = sum_n h3[n, h] * (batch[n] == g) ----
        pool_ps = psP.tile([H, G], F32)
        for b in range(a_blk):
            tp = psT.tile([128, 128], F32, tag="tp", name=f"tpp_{b}")
            nc.tensor.transpose(tp[:], h3T_sb[:, b * 128 : (b + 1) * 128],
                                ident_sb[:])
            nm = nm_pool.tile([128, 128], F32, tag="nm", name=f"nmp_{b}")
            nc.scalar.copy(nm[:], tp[:])
            pb = m_pool.tile([128, 128], F32, tag="m", name=f"pb_{b}")
            nc.vector.tensor_scalar(pb[:], iota_sb[:],
                                    batchrel_sb[:, b : b + 1], None,
                                    ALU.is_equal)
            nc.tensor.matmul(pool_ps[:], nm[:], pb[:],
                             start=(b == 0), stop=(b == a_blk - 1))
        poolT_sb = sm_pool.tile([H, G], F32)
        nc.vector.tensor_copy(poolT_sb[:], pool_ps[:])
        nc.sync.dma_start(pool_in[:], poolT_sb[:])
        nc.gpsimd.collective_compute(
            "AllReduce", ALU.add, replica_groups=[list(range(NCORES))],
            ins=[pool_in.opt()], outs=[pool_out.opt()])
        poolT_full = sm_pool.tile([H, G], F32)
        nc.sync.dma_start(poolT_full[:], pool_out[:])

        # ---- logits = pooled @ w_out + b_out, then log_softmax ----
        log_ps = psB.tile([H, 512], F32, tag="hp", name="log_ps")
        nc.tensor.matmul(log_ps[:G, :C_OUT], poolT_full[:], wout_sb[:],
                         start=True, stop=False)
        nc.tensor.matmul(log_ps[:G, :C_OUT], ones_sb[:], bout_sb[:],
                         start=False, stop=True)
        logits = sm_pool.tile([G, C_OUT], F32)
        nc.vector.tensor_copy(logits[:], log_ps[:G, :C_OUT])
        mx = sm_pool.tile([G, 1], F32)
        nc.vector.tensor_reduce(mx[:], logits[:], mybir.AxisListType.X,
                                ALU.max)
        negmx = sm_pool.tile([G, 1], F32)
        nc.scalar.mul(negmx[:], mx[:], -1.0)
        expv = sm_pool.tile([G, C_OUT], F32)
        nc.scalar.activation(expv[:], logits[:], AF.Exp, bias=negmx[:])
        sm = sm_pool.tile([G, 1], F32)
        nc.vector.tensor_reduce(sm[:], expv[:], mybir.AxisListType.X, ALU.add)
        lse = sm_pool.tile([G, 1], F32)
        nc.scalar.activation(lse[:], sm[:], AF.Ln)
        mxlse = sm_pool.tile([G, 1], F32)
        nc.vector.tensor_add(mxlse[:], mx[:], lse[:])
        outv = sm_pool.tile([G, C_OUT], F32)
        nc.vector.tensor_scalar(outv[:], logits[:], mxlse[:], None,
                                ALU.subtract)
        nc.sync.dma_start(out_t[:], outv[:])

    nc.compile()
    return nc


_CACHE = {}


def kernel(x, edge_index, batch, w1_rel, b1, w1_root, w2_rel, b2, w2_root,
           w3_rel, b3, w3_root, w_out, b_out):
    in_maps, cchunks = _prep_inputs(x, edge_index, batch)
    weights = {
        "w1_rel": np.asarray(w1_rel, np.float32),
        "w1_root": np.asarray(w1_root, np.float32),
        "w2_rel": np.asarray(w2_rel, np.float32),
        "w2_root": np.asarray(w2_root, np.float32),
        "w3_rel": np.asarray(w3_rel, np.float32),
        "w3_root": np.asarray(w3_root, np.float32),
        "b1": np.asarray(b1, np.float32).reshape(H, 1),
        "b2": np.asarray(b2, np.float32).reshape(H, 1),
        "b3": np.asarray(b3, np.float32).reshape(H, 1),
        "w_out": np.asarray(w_out, np.float32),
        "b_out": np.asarray(b_out, np.float32).reshape(1, C_OUT),
    }
    for m in in_maps:
        m.update(weights)

    if cchunks not in _CACHE:
        _CACHE[cchunks] = _build_program(cchunks)
    nc = _CACHE[cchunks]
    res = run_bass_kernel_spmd(nc, in_maps, core_ids=list(range(NCORES)))
    return np.asarray(res.results[0]["out"], np.float32)



# revision 4
# speedup vs baseline: 2.4488x; 2.4488x over previous
"""Distributed 3-layer GraphConv GNN on 8 Trainium2 NeuronCores — v2.

Sharding: nodes (and their incoming edges) partitioned contiguously across
8 cores (2560 nodes / 20 blocks of 128 dst nodes per core). Per layer each
core dma_gathers the source-node rows for its edges (sorted by dst, then by
src within each block for HBM locality), segment-sums them per dst block on
the TensorEngine via one-hot matmuls, applies the dense transform + bias +
ReLU, and AllGathers the bf16 node-major features for the next layer.

v2 vs v1:
  - bf16 feature payloads and matmuls (tolerance is 2e-2; bf16 keeps ~1e-3):
    4x TensorE throughput, half the gather/collective bytes.
  - x zero-padded to 128 features host-side; all 3 layers share one 256B-row
    bf16 gather path (W1 padded with zero rows to match).
  - one batched DVE tensor_tensor (broadcast APs) builds a whole block's
    one-hot masks instead of one tensor_scalar per 128-edge chunk.
  - Shared-output collectives (fast path), bf16 AllGather payloads.
  - minimal host inputs (~1.1MB/core vs ~8MB): x fed sharded and AllGathered
    on device; gather indices fed 16-wide and replicated on device; iota and
    identity built on device.
"""
import sys

sys.path.insert(0, "/opt/trn_rl_repo")

from contextlib import ExitStack

import numpy as np
import ml_dtypes

import concourse.bass as bass
import concourse.tile as tile
from concourse import bacc, mybir
from concourse.bass_utils import run_bass_kernel_spmd
from concourse.library_config import mlp as mlp_lib

N, E, F_IN, H, C_OUT, G = 20000, 640000, 64, 128, 10, 128
NCORES = 8
NPC = 2560            # nodes per core
NBLK = NPC // 128     # dst blocks per core (20)
NBLK_GLOB = NCORES * NBLK  # 160
NPAD = NCORES * NPC   # 20480
F32 = mybir.dt.float32
BF16 = mybir.dt.bfloat16
I16 = mybir.dt.int16
I8 = mybir.dt.int8
AF = mybir.ActivationFunctionType
ALU = mybir.AluOpType

GCH = 8  # chunks (x128 idxs) per dma_gather call; 1024 idxs max per call


def _prep_inputs(x, edge_index, batch):
    """Host-side edge partitioning/padding. Returns per-core input dicts
    (minus weights) and the uniform chunks-per-block count."""
    src = np.asarray(edge_index[0], dtype=np.int64)
    dst = np.asarray(edge_index[1], dtype=np.int64)
    batch = np.asarray(batch, dtype=np.int64)
    x = np.ascontiguousarray(np.asarray(x, dtype=np.float32))

    order = np.argsort(dst, kind="stable")
    dst_s = dst[order]
    src_s = src[order]
    starts = np.searchsorted(dst_s, np.arange(NBLK_GLOB) * 128)
    ends = np.searchsorted(dst_s, (np.arange(NBLK_GLOB) + 1) * 128)
    counts = ends - starts
    cchunks = max(1, int(np.ceil(counts.max() / 128)))
    L = cchunks * 128  # padded edges per block

    in_maps = []
    for k in range(NCORES):
        src_pad = np.zeros((NBLK, L), np.int64)
        dstrel_pad = np.full((NBLK, L), -1, np.int64)
        for b in range(NBLK):
            gb = k * NBLK + b
            s, e = starts[gb], ends[gb]
            n = e - s
            if n:
                # sort by src within the block for gather locality
                o = np.argsort(src_s[s:e], kind="stable")
                src_pad[b, :n] = src_s[s:e][o]
                dstrel_pad[b, :n] = dst_s[s:e][o] - gb * 128
        # dma_gather idx layout: idx i at [i % 16, i // 16]; fed 16-wide,
        # replicated to the 8 groups of 16 partitions on device.
        idx16 = np.concatenate(
            list(src_pad.reshape(NBLK, L // 16, 16).transpose(0, 2, 1)),
            axis=1).astype(np.int16)  # [16, NBLK*L/16]
        # dstrel layout for batched one-hot: edge e = c*128 + p of block b
        # at [p, b*cchunks + c]; int8 (-1..127)
        dr = dstrel_pad.reshape(NBLK, cchunks, 128).transpose(0, 2, 1)
        dr_t = np.ascontiguousarray(np.concatenate(list(dr), axis=1)).astype(
            np.int8)

        # graph id per local node, [128, NBLK] int8; -1 for pad nodes
        base = k * NPC
        valid = max(0, min(NPC, N - base))
        flat = np.full(NPC, -1, np.int64)
        if valid:
            flat[:valid] = batch[base:base + valid]
        batchrel_t = np.ascontiguousarray(
            flat.reshape(NBLK, 128).T).astype(np.int8)

        # local node-major padded bf16 x slice
        x_loc = np.zeros((NPC, H), np.float32)
        if valid:
            x_loc[:valid, :F_IN] = x[base:base + valid]
        in_maps.append(
            {
                "x_loc": x_loc.astype(ml_dtypes.bfloat16),
                "idx16_t": idx16,
                "dstrel_t": dr_t,
                "batchrel_t": batchrel_t,
                "iota_t": np.tile(np.arange(128, dtype=np.int8), (128, 1)),
                "ident_t": np.eye(128, dtype=np.int8),
            }
        )
    return in_maps, cchunks


def _build_program(cchunks):
    L = cchunks * 128
    nc = bacc.Bacc("TRN2", target_bir_lowering=False, debug=False,
                   num_devices=NCORES)

    x_loc_in = nc.dram_tensor("x_loc", [NPC, H], BF16, kind="ExternalInput")
    iota_t = nc.dram_tensor("iota_t", [128, 128], I8, kind="ExternalInput")
    ident_t = nc.dram_tensor("ident_t", [128, 128], I8, kind="ExternalInput")
    idx16_t = nc.dram_tensor("idx16_t", [16, NBLK * L // 16], I16,
                             kind="ExternalInput")
    dstrel_t = nc.dram_tensor("dstrel_t", [128, NBLK * cchunks], I8,
                              kind="ExternalInput")
    batchrel_t = nc.dram_tensor("batchrel_t", [128, NBLK], I8,
                                kind="ExternalInput")
    w_rel_in = [nc.dram_tensor(f"w{i}_rel", [H, H], BF16,
                               kind="ExternalInput") for i in (1, 2, 3)]
    w_root_in = [nc.dram_tensor(f"w{i}_root", [H, H], BF16,
                                kind="ExternalInput") for i in (1, 2, 3)]
    b_in = [nc.dram_tensor(f"b{i}", [H, 1], F32, kind="ExternalInput")
            for i in (1, 2, 3)]
    w_out_in = nc.dram_tensor("w_out", [H, C_OUT], F32, kind="ExternalInput")
    b_out_in = nc.dram_tensor("b_out", [1, C_OUT], F32, kind="ExternalInput")
    out_t = nc.dram_tensor("out", [G, C_OUT], F32, kind="ExternalOutput")

    with tile.TileContext(nc) as tc, ExitStack() as ctx:
        const = ctx.enter_context(tc.tile_pool(name="const", bufs=1))
        feat = ctx.enter_context(tc.tile_pool(name="feat", bufs=1))
        xe_pool = ctx.enter_context(tc.tile_pool(name="xe", bufs=3))
        m_pool = ctx.enter_context(tc.tile_pool(name="m", bufs=3))
        nm_pool = ctx.enter_context(tc.tile_pool(name="nm", bufs=4))
        sm_pool = ctx.enter_context(tc.tile_pool(name="sm", bufs=1))
        psA = ctx.enter_context(tc.tile_pool(name="psA", bufs=2,
                                             space="PSUM"))
        psB = ctx.enter_context(tc.tile_pool(name="psB", bufs=2,
                                             space="PSUM"))
        psT = ctx.enter_context(tc.tile_pool(name="psT", bufs=2,
                                             space="PSUM"))
        psP = ctx.enter_context(tc.tile_pool(name="psP", bufs=1,
                                             space="PSUM"))
        dram = ctx.enter_context(tc.tile_pool(name="dram", bufs=1,
                                              space="DRAM"))

        nc.gpsimd.load_library(mlp_lib)

        # ---- small constants (int8 inputs, converted on device) ----
        iota_i8 = const.tile([128, 128], I8)
        nc.sync.dma_start(iota_i8[:], iota_t[:])
        iota_sb = const.tile([128, 128], BF16)
        nc.vector.tensor_copy(iota_sb[:], iota_i8[:])
        ident_i8 = const.tile([128, 128], I8)
        nc.sync.dma_start(ident_i8[:], ident_t[:])
        ident_sb = const.tile([128, 128], BF16)
        nc.vector.tensor_copy(ident_sb[:], ident_i8[:])
        ones_sb = const.tile([1, 128], F32)
        nc.vector.memset(ones_sb[:], 1.0)

        # ---- inputs to SBUF ----
        idx_sb = const.tile([128, NBLK * L // 16], I16)
        for gseg in range(8):
            nc.sync.dma_start(idx_sb[gseg * 16:(gseg + 1) * 16, :],
                              idx16_t[:])
        dstrel_i8 = const.tile([128, NBLK * cchunks], I8)
        nc.sync.dma_start(dstrel_i8[:], dstrel_t[:])
        dstrel_sb = const.tile([128, NBLK * cchunks], BF16)
        nc.vector.tensor_copy(dstrel_sb[:], dstrel_i8[:])
        batchrel_i8 = const.tile([128, NBLK], I8)
        nc.sync.dma_start(batchrel_i8[:], batchrel_t[:])
        batchrel_sb = const.tile([128, NBLK], F32)
        nc.vector.tensor_copy(batchrel_sb[:], batchrel_i8[:])
        w_rel_sb, w_root_sb, b_sb = [], [], []
        for i in range(3):
            wr = const.tile([H, H], BF16, name=f"wrel{i}")
            nc.sync.dma_start(wr[:], w_rel_in[i][:])
            w_rel_sb.append(wr)
            wo = const.tile([H, H], BF16, name=f"wroot{i}")
            nc.sync.dma_start(wo[:], w_root_in[i][:])
            w_root_sb.append(wo)
            bb = const.tile([H, 1], F32, name=f"b{i}")
            nc.sync.dma_start(bb[:], b_in[i][:])
            b_sb.append(bb)
        wout_sb = const.tile([H, C_OUT], F32)
        nc.sync.dma_start(wout_sb[:], w_out_in[:])
        bout_sb = const.tile([1, C_OUT], F32)
        nc.sync.dma_start(bout_sb[:], b_out_in[:])

        # ---- feature-major tiles ----
        xT_sb = feat.tile([H, NPC], BF16)
        h1T_sb = feat.tile([H, NPC], BF16)
        h2T_sb = feat.tile([H, NPC], BF16)
        h3T_sb = feat.tile([H, NPC], BF16)
        aggT_sb = feat.tile([H, NPC], BF16)

        x_stage = dram.tile([NPC, H], BF16)
        h1_loc = dram.tile([NPC, H], BF16)
        h2_loc = dram.tile([NPC, H], BF16)
        x_full = dram.tile([NPAD, H], BF16, addr_space="Shared")
        h1_full = dram.tile([NPAD, H], BF16, addr_space="Shared")
        h2_full = dram.tile([NPAD, H], BF16, addr_space="Shared")
        pool_in = dram.tile([H, G], F32)
        pool_out = dram.tile([H, G], F32, addr_space="Shared")

        # ---- AllGather x; derive feature-major xT from node-major x ----
        nc.sync.dma_start(x_stage[:], x_loc_in[:])
        nc.gpsimd.collective_compute(
            "AllGather", ALU.bypass, replica_groups=[list(range(NCORES))],
            ins=[x_stage.opt()], outs=[x_full.opt()])
        for b in range(NBLK):
            xb = nm_pool.tile([128, 128], BF16, tag="nm", name=f"xb_{b}")
            nc.sync.dma_start(xb[:], x_loc_in[b * 128:(b + 1) * 128, :])
            tp = psT.tile([128, 128], BF16, tag="tp", name=f"xtp_{b}")
            nc.tensor.transpose(tp[:], xb[:], ident_sb[:])
            nc.scalar.copy(xT_sb[:, b * 128:(b + 1) * 128], tp[:])

        def gcn_layer(li, gather_src, inT_sb, outT_sb, h_loc, h_full):
            wrel, wroot, bb = w_rel_sb[li], w_root_sb[li], b_sb[li]
            for b in range(NBLK):
                xe = xe_pool.tile([128, cchunks * H], BF16, tag="xe",
                                  name=f"xe{li}_{b}")
                for g0 in range(0, cchunks, GCH):
                    g1 = min(g0 + GCH, cchunks)
                    nsub = (g1 - g0) * 128
                    xe3 = xe[:, g0 * H:g1 * H].rearrange(
                        "p (c f) -> p c f", f=H)
                    nc.gpsimd.dma_gather(
                        xe3, gather_src[:],
                        idx_sb[:, b * (L // 16) + g0 * 8
                               : b * (L // 16) + g0 * 8 + nsub // 16],
                        nsub, nsub, H)
                m = m_pool.tile([128, cchunks * 128], BF16, tag="m",
                                name=f"m{li}_{b}")
                m3 = m[:].rearrange("p (c d) -> p c d", c=cchunks)
                iota_b = iota_sb[:].rearrange("p (o d) -> p o d", o=1) \
                    .broadcast_to([128, cchunks, 128])
                dst_b = dstrel_sb[:, b * cchunks:(b + 1) * cchunks] \
                    .rearrange("p (c o) -> p c o", o=1) \
                    .broadcast_to([128, cchunks, 128])
                nc.vector.tensor_tensor(m3, iota_b, dst_b, ALU.is_equal)
                agg_ps = psA.tile([128, 128], F32, tag="agg",
                                  name=f"agg{li}_{b}")
                for c in range(cchunks):
                    nc.tensor.matmul(
                        agg_ps[:], xe[:, c * H:(c + 1) * H],
                        m[:, c * 128:(c + 1) * 128],
                        start=(c == 0), stop=(c == cchunks - 1))
                nc.vector.tensor_copy(
                    aggT_sb[:, b * 128:(b + 1) * 128], agg_ps[:])
            # dense transform + bias + relu (feature-major, bf16 out)
            for g in range(NPC // 512):
                hp = psB.tile([H, 512], F32, tag="hp", name=f"hp{li}_{g}")
                nc.tensor.matmul(hp[:], wrel[:],
                                 aggT_sb[:, g * 512:(g + 1) * 512],
                                 start=True, stop=False)
                nc.tensor.matmul(hp[:], wroot[:],
                                 inT_sb[:, g * 512:(g + 1) * 512],
                                 start=False, stop=True)
                nc.scalar.activation(outT_sb[:, g * 512:(g + 1) * 512],
                                     hp[:], AF.Relu, bias=bb[:])
            # node-major store + allgather for next layer's gather source
            if h_loc is not None:
                for b in range(NBLK):
                    tp = psT.tile([128, 128], BF16, tag="tp",
                                  name=f"tp{li}_{b}")
                    nc.tensor.transpose(
                        tp[:], outT_sb[:, b * 128:(b + 1) * 128],
                        ident_sb[:])
                    nm = nm_pool.tile([128, 128], BF16, tag="nm",
                                      name=f"nm{li}_{b}")
                    nc.scalar.copy(nm[:], tp[:])
                    nc.sync.dma_start(h_loc[b * 128:(b + 1) * 128, :],
                                      nm[:])
                nc.gpsimd.collective_compute(
                    "AllGather", ALU.bypass,
                    replica_groups=[list(range(NCORES))],
                    ins=[h_loc.opt()], outs=[h_full.opt()])

        gcn_layer(0, x_full, xT_sb, h1T_sb, h1_loc, h1_full)
        gcn_layer(1, h1_full, h1T_sb, h2T_sb, h2_loc, h2_full)
        gcn_layer(2, h2_full, h2T_sb, h3T_sb, None, None)

        # ---- pooling: pooledT[h, g] = sum_n h3[n, h] * (batch[n] == g) ----
        pool_ps = psP.tile([H, G], F32)
        for b in range(NBLK):
            tp = psT.tile([128, 128], BF16, tag="tp", name=f"tpp_{b}")
            nc.tensor.transpose(tp[:], h3T_sb[:, b * 128:(b + 1) * 128],
                                ident_sb[:])
            nm = nm_pool.tile([128, 128], BF16, tag="nm", name=f"nmp_{b}")
            nc.scalar.copy(nm[:], tp[:])
            pb = m_pool.tile([128, 128], BF16, tag="m", name=f"pb_{b}")
            nc.vector.tensor_scalar(pb[:], iota_sb[:],
                                    batchrel_sb[:, b:b + 1], None,
                                    ALU.is_equal)
            nc.tensor.matmul(pool_ps[:], nm[:], pb[:],
                             start=(b == 0), stop=(b == NBLK - 1))
        poolT_sb = sm_pool.tile([H, G], F32)
        nc.vector.tensor_copy(poolT_sb[:], pool_ps[:])
        nc.sync.dma_start(pool_in[:], poolT_sb[:])
        nc.gpsimd.collective_compute(
            "AllReduce", ALU.add, replica_groups=[list(range(NCORES))],
            ins=[pool_in.opt()], outs=[pool_out.opt()])
        poolT_full = sm_pool.tile([H, G], F32)
        nc.sync.dma_start(poolT_full[:], pool_out[:])

        # ---- logits = pooled @ w_out + b_out, then log_softmax ----
        log_ps = psB.tile([H, 512], F32, tag="hp", name="log_ps")
        nc.tensor.matmul(log_ps[:G, :C_OUT], poolT_full[:], wout_sb[:],
                         start=True, stop=False)
        nc.tensor.matmul(log_ps[:G, :C_OUT], ones_sb[:], bout_sb[:],
                         start=False, stop=True)
        logits = sm_pool.tile([G, C_OUT], F32)
        nc.vector.tensor_copy(logits[:], log_ps[:G, :C_OUT])
        mx = sm_pool.tile([G, 1], F32)
        nc.vector.tensor_reduce(mx[:], logits[:], mybir.AxisListType.X,
                                ALU.max)
        negmx = sm_pool.tile([G, 1], F32)
        nc.scalar.mul(negmx[:], mx[:], -1.0)
        expv = sm_pool.tile([G, C_OUT], F32)
        nc.scalar.activation(expv[:], logits[:], AF.Exp, bias=negmx[:])
        sm = sm_pool.tile([G, 1], F32)
        nc.vector.tensor_reduce(sm[:], expv[:], mybir.AxisListType.X,
                                ALU.add)
        lse = sm_pool.tile([G, 1], F32)
        nc.scalar.activation(lse[:], sm[:], AF.Ln)
        mxlse = sm_pool.tile([G, 1], F32)
        nc.vector.tensor_add(mxlse[:], mx[:], lse[:])
        outv = sm_pool.tile([G, C_OUT], F32)
        nc.vector.tensor_scalar(outv[:], logits[:], mxlse[:], None,
                                ALU.subtract)
        nc.sync.dma_start(out_t[:], outv[:])

    nc.compile()
    return nc


_CACHE = {}


def _weights_map(w1_rel, b1, w1_root, w2_rel, b2, w2_root, w3_rel, b3,
                 w3_root, w_out, b_out):
    def pad128(w):
        w = np.asarray(w, np.float32)
        if w.shape[0] < H:
            w = np.concatenate(
                [w, np.zeros((H - w.shape[0], w.shape[1]), np.float32)],
                axis=0)
        return w.astype(ml_dtypes.bfloat16)

    return {
        "w1_rel": pad128(w1_rel),
        "w1_root": pad128(w1_root),
        "w2_rel": pad128(w2_rel),
        "w2_root": pad128(w2_root),
        "w3_rel": pad128(w3_rel),
        "w3_root": pad128(w3_root),
        "b1": np.asarray(b1, np.float32).reshape(H, 1),
        "b2": np.asarray(b2, np.float32).reshape(H, 1),
        "b3": np.asarray(b3, np.float32).reshape(H, 1),
        "w_out": np.asarray(w_out, np.float32),
        "b_out": np.asarray(b_out, np.float32).reshape(1, C_OUT),
    }


def kernel(x, edge_index, batch, w1_rel, b1, w1_root, w2_rel, b2, w2_root,
           w3_rel, b3, w3_root, w_out, b_out):
    in_maps, cchunks = _prep_inputs(x, edge_index, batch)
    weights = _weights_map(w1_rel, b1, w1_root, w2_rel, b2, w2_root,
                           w3_rel, b3, w3_root, w_out, b_out)
    for m in in_maps:
        m.update(weights)

    if cchunks not in _CACHE:
        _CACHE[cchunks] = _build_program(cchunks)
    nc = _CACHE[cchunks]
    res = run_bass_kernel_spmd(nc, in_maps, core_ids=list(range(NCORES)))
    return np.asarray(res.results[0]["out"], np.float32)


# revision 5
# speedup vs baseline: 2.5279x; 1.0323x over previous
"""Distributed 3-layer GraphConv GNN on 8 Trainium2 NeuronCores — v2.

Sharding: nodes (and their incoming edges) partitioned contiguously across
8 cores (2560 nodes / 20 blocks of 128 dst nodes per core). Per layer each
core dma_gathers the source-node rows for its edges (sorted by dst, then by
src within each block for HBM locality), segment-sums them per dst block on
the TensorEngine via one-hot matmuls, applies the dense transform + bias +
ReLU, and AllGathers the bf16 node-major features for the next layer.

v2 vs v1:
  - bf16 feature payloads and matmuls (tolerance is 2e-2; bf16 keeps ~1e-3):
    4x TensorE throughput, half the gather/collective bytes.
  - x zero-padded to 128 features host-side; all 3 layers share one 256B-row
    bf16 gather path (W1 padded with zero rows to match).
  - one batched DVE tensor_tensor (broadcast APs) builds a whole block's
    one-hot masks instead of one tensor_scalar per 128-edge chunk.
  - Shared-output collectives (fast path), bf16 AllGather payloads.
  - minimal host inputs (~1.1MB/core vs ~8MB): x fed sharded and AllGathered
    on device; gather indices fed 16-wide and replicated on device; iota and
    identity built on device.
"""
import sys

sys.path.insert(0, "/opt/trn_rl_repo")

from contextlib import ExitStack

import numpy as np
import ml_dtypes

import concourse.bass as bass
import concourse.tile as tile
from concourse import bacc, mybir
from concourse.bass_utils import run_bass_kernel_spmd
from concourse.library_config import mlp as mlp_lib

N, E, F_IN, H, C_OUT, G = 20000, 640000, 64, 128, 10, 128
NCORES = 8
NPC = 2560            # nodes per core
NBLK = NPC // 128     # dst blocks per core (20)
NBLK_GLOB = NCORES * NBLK  # 160
NPAD = NCORES * NPC   # 20480
F32 = mybir.dt.float32
BF16 = mybir.dt.bfloat16
I16 = mybir.dt.int16
I8 = mybir.dt.int8
AF = mybir.ActivationFunctionType
ALU = mybir.AluOpType

GCH = 8  # chunks (x128 idxs) per dma_gather call; 1024 idxs max per call


def _prep_inputs(x, edge_index, batch):
    """Host-side edge partitioning/padding. Returns per-core input dicts
    (minus weights) and the uniform chunks-per-block count."""
    src = np.asarray(edge_index[0], dtype=np.int64)
    dst = np.asarray(edge_index[1], dtype=np.int64)
    batch = np.asarray(batch, dtype=np.int64)
    x = np.ascontiguousarray(np.asarray(x, dtype=np.float32))

    order = np.argsort(dst, kind="stable")
    dst_s = dst[order]
    src_s = src[order]
    starts = np.searchsorted(dst_s, np.arange(NBLK_GLOB) * 128)
    ends = np.searchsorted(dst_s, (np.arange(NBLK_GLOB) + 1) * 128)
    counts = ends - starts
    cchunks = max(1, int(np.ceil(counts.max() / 128)))
    L = cchunks * 128  # padded edges per block

    x_pad = np.zeros((NPAD, H), np.float32)
    x_pad[:N, :F_IN] = x
    x_pad_bf = x_pad.astype(ml_dtypes.bfloat16)

    in_maps = []
    for k in range(NCORES):
        src_pad = np.zeros((NBLK, L), np.int64)
        dstrel_pad = np.full((NBLK, L), -1, np.int64)
        for b in range(NBLK):
            gb = k * NBLK + b
            s, e = starts[gb], ends[gb]
            n = e - s
            if n:
                # sort by src within the block for gather locality
                o = np.argsort(src_s[s:e], kind="stable")
                src_pad[b, :n] = src_s[s:e][o]
                dstrel_pad[b, :n] = dst_s[s:e][o] - gb * 128
        # dma_gather idx layout: idx i at [i % 16, i // 16]; fed 16-wide,
        # replicated to the 8 groups of 16 partitions on device.
        idx16 = np.concatenate(
            list(src_pad.reshape(NBLK, L // 16, 16).transpose(0, 2, 1)),
            axis=1).astype(np.int16)  # [16, NBLK*L/16]
        # dstrel layout for batched one-hot: edge e = c*128 + p of block b
        # at [p, b*cchunks + c]; int8 (-1..127)
        dr = dstrel_pad.reshape(NBLK, cchunks, 128).transpose(0, 2, 1)
        dr_t = np.ascontiguousarray(np.concatenate(list(dr), axis=1)).astype(
            np.int8)

        # graph id per local node, [128, NBLK] int8; -1 for pad nodes
        base = k * NPC
        valid = max(0, min(NPC, N - base))
        flat = np.full(NPC, -1, np.int64)
        if valid:
            flat[:valid] = batch[base:base + valid]
        batchrel_t = np.ascontiguousarray(
            flat.reshape(NBLK, 128).T).astype(np.int8)

        # local node-major padded bf16 x slice
        x_loc = np.zeros((NPC, H), np.float32)
        if valid:
            x_loc[:valid, :F_IN] = x[base:base + valid]
        in_maps.append(
            {
                "x_loc": x_loc.astype(ml_dtypes.bfloat16),
                "x_full": x_pad_bf,
                "idx16_t": idx16,
                "dstrel_t": dr_t,
                "batchrel_t": batchrel_t,
                "iota_t": np.tile(np.arange(128, dtype=np.int8), (128, 1)),
                "ident_t": np.eye(128, dtype=np.int8),
            }
        )
    return in_maps, cchunks


def _build_program(cchunks):
    L = cchunks * 128
    nc = bacc.Bacc("TRN2", target_bir_lowering=False, debug=False,
                   num_devices=NCORES)

    x_loc_in = nc.dram_tensor("x_loc", [NPC, H], BF16, kind="ExternalInput")
    x_full_in = nc.dram_tensor("x_full", [NPAD, H], BF16,
                               kind="ExternalInput")
    iota_t = nc.dram_tensor("iota_t", [128, 128], I8, kind="ExternalInput")
    ident_t = nc.dram_tensor("ident_t", [128, 128], I8, kind="ExternalInput")
    idx16_t = nc.dram_tensor("idx16_t", [16, NBLK * L // 16], I16,
                             kind="ExternalInput")
    dstrel_t = nc.dram_tensor("dstrel_t", [128, NBLK * cchunks], I8,
                              kind="ExternalInput")
    batchrel_t = nc.dram_tensor("batchrel_t", [128, NBLK], I8,
                                kind="ExternalInput")
    w_rel_in = [nc.dram_tensor(f"w{i}_rel", [H, H], BF16,
                               kind="ExternalInput") for i in (1, 2, 3)]
    w_root_in = [nc.dram_tensor(f"w{i}_root", [H, H], BF16,
                                kind="ExternalInput") for i in (1, 2, 3)]
    b_in = [nc.dram_tensor(f"b{i}", [H, 1], F32, kind="ExternalInput")
            for i in (1, 2, 3)]
    w_out_in = nc.dram_tensor("w_out", [H, C_OUT], F32, kind="ExternalInput")
    b_out_in = nc.dram_tensor("b_out", [1, C_OUT], F32, kind="ExternalInput")
    out_t = nc.dram_tensor("out", [G, C_OUT], F32, kind="ExternalOutput")

    with tile.TileContext(nc) as tc, ExitStack() as ctx:
        const = ctx.enter_context(tc.tile_pool(name="const", bufs=1))
        feat = ctx.enter_context(tc.tile_pool(name="feat", bufs=1))
        xe_pool = ctx.enter_context(tc.tile_pool(name="xe", bufs=3))
        m_pool = ctx.enter_context(tc.tile_pool(name="m", bufs=3))
        nm_pool = ctx.enter_context(tc.tile_pool(name="nm", bufs=4))
        sm_pool = ctx.enter_context(tc.tile_pool(name="sm", bufs=1))
        psA = ctx.enter_context(tc.tile_pool(name="psA", bufs=2,
                                             space="PSUM"))
        psB = ctx.enter_context(tc.tile_pool(name="psB", bufs=2,
                                             space="PSUM"))
        psT = ctx.enter_context(tc.tile_pool(name="psT", bufs=2,
                                             space="PSUM"))
        psP = ctx.enter_context(tc.tile_pool(name="psP", bufs=1,
                                             space="PSUM"))
        dram = ctx.enter_context(tc.tile_pool(name="dram", bufs=1,
                                              space="DRAM"))

        nc.gpsimd.load_library(mlp_lib)

        # ---- small constants (int8 inputs, converted on device) ----
        iota_i8 = const.tile([128, 128], I8)
        nc.sync.dma_start(iota_i8[:], iota_t[:])
        iota_sb = const.tile([128, 128], BF16)
        nc.vector.tensor_copy(iota_sb[:], iota_i8[:])
        ident_i8 = const.tile([128, 128], I8)
        nc.sync.dma_start(ident_i8[:], ident_t[:])
        ident_sb = const.tile([128, 128], BF16)
        nc.vector.tensor_copy(ident_sb[:], ident_i8[:])
        ones_sb = const.tile([1, 128], F32)
        nc.vector.memset(ones_sb[:], 1.0)

        # ---- inputs to SBUF ----
        idx_sb = const.tile([128, NBLK * L // 16], I16)
        for gseg in range(8):
            nc.sync.dma_start(idx_sb[gseg * 16:(gseg + 1) * 16, :],
                              idx16_t[:])
        dstrel_i8 = const.tile([128, NBLK * cchunks], I8)
        nc.sync.dma_start(dstrel_i8[:], dstrel_t[:])
        dstrel_sb = const.tile([128, NBLK * cchunks], BF16)
        nc.vector.tensor_copy(dstrel_sb[:], dstrel_i8[:])
        batchrel_i8 = const.tile([128, NBLK], I8)
        nc.sync.dma_start(batchrel_i8[:], batchrel_t[:])
        batchrel_sb = const.tile([128, NBLK], F32)
        nc.vector.tensor_copy(batchrel_sb[:], batchrel_i8[:])
        w_rel_sb, w_root_sb, b_sb = [], [], []
        for i in range(3):
            wr = const.tile([H, H], BF16, name=f"wrel{i}")
            nc.sync.dma_start(wr[:], w_rel_in[i][:])
            w_rel_sb.append(wr)
            wo = const.tile([H, H], BF16, name=f"wroot{i}")
            nc.sync.dma_start(wo[:], w_root_in[i][:])
            w_root_sb.append(wo)
            bb = const.tile([H, 1], F32, name=f"b{i}")
            nc.sync.dma_start(bb[:], b_in[i][:])
            b_sb.append(bb)
        wout_sb = const.tile([H, C_OUT], F32)
        nc.sync.dma_start(wout_sb[:], w_out_in[:])
        bout_sb = const.tile([1, C_OUT], F32)
        nc.sync.dma_start(bout_sb[:], b_out_in[:])

        # ---- feature-major tiles ----
        xT_sb = feat.tile([H, NPC], BF16)
        h1T_sb = feat.tile([H, NPC], BF16)
        h2T_sb = feat.tile([H, NPC], BF16)
        h3T_sb = feat.tile([H, NPC], BF16)
        aggT_sb = feat.tile([H, NPC], BF16)

        h1_loc = dram.tile([NPC, H], BF16)
        h2_loc = dram.tile([NPC, H], BF16)
        h1_full = dram.tile([NPAD, H], BF16, addr_space="Shared")
        h2_full = dram.tile([NPAD, H], BF16, addr_space="Shared")
        pool_in = dram.tile([H, G], F32)
        pool_out = dram.tile([H, G], F32, addr_space="Shared")

        # ---- derive feature-major xT from node-major x ----
        for b in range(NBLK):
            xb = nm_pool.tile([128, 128], BF16, tag="nm", name=f"xb_{b}")
            nc.sync.dma_start(xb[:], x_loc_in[b * 128:(b + 1) * 128, :])
            tp = psT.tile([128, 128], BF16, tag="tp", name=f"xtp_{b}")
            nc.tensor.transpose(tp[:], xb[:], ident_sb[:])
            nc.scalar.copy(xT_sb[:, b * 128:(b + 1) * 128], tp[:])

        def gcn_layer(li, gather_src, inT_sb, outT_sb, h_loc, h_full):
            wrel, wroot, bb = w_rel_sb[li], w_root_sb[li], b_sb[li]
            for b in range(NBLK):
                xe = xe_pool.tile([128, cchunks * H], BF16, tag="xe",
                                  name=f"xe{li}_{b}")
                for g0 in range(0, cchunks, GCH):
                    g1 = min(g0 + GCH, cchunks)
                    nsub = (g1 - g0) * 128
                    xe3 = xe[:, g0 * H:g1 * H].rearrange(
                        "p (c f) -> p c f", f=H)
                    nc.gpsimd.dma_gather(
                        xe3, gather_src[:],
                        idx_sb[:, b * (L // 16) + g0 * 8
                               : b * (L // 16) + g0 * 8 + nsub // 16],
                        nsub, nsub, H)
                m = m_pool.tile([128, cchunks * 128], BF16, tag="m",
                                name=f"m{li}_{b}")
                m3 = m[:].rearrange("p (c d) -> p c d", c=cchunks)
                iota_b = iota_sb[:].rearrange("p (o d) -> p o d", o=1) \
                    .broadcast_to([128, cchunks, 128])
                dst_b = dstrel_sb[:, b * cchunks:(b + 1) * cchunks] \
                    .rearrange("p (c o) -> p c o", o=1) \
                    .broadcast_to([128, cchunks, 128])
                nc.vector.tensor_tensor(m3, iota_b, dst_b, ALU.is_equal)
                agg_ps = psA.tile([128, 128], F32, tag="agg",
                                  name=f"agg{li}_{b}")
                for c in range(cchunks):
                    nc.tensor.matmul(
                        agg_ps[:], xe[:, c * H:(c + 1) * H],
                        m[:, c * 128:(c + 1) * 128],
                        start=(c == 0), stop=(c == cchunks - 1))
                nc.vector.tensor_copy(
                    aggT_sb[:, b * 128:(b + 1) * 128], agg_ps[:])
            # dense transform + bias + relu (feature-major, bf16 out)
            for g in range(NPC // 512):
                hp = psB.tile([H, 512], F32, tag="hp", name=f"hp{li}_{g}")
                nc.tensor.matmul(hp[:], wrel[:],
                                 aggT_sb[:, g * 512:(g + 1) * 512],
                                 start=True, stop=False)
                nc.tensor.matmul(hp[:], wroot[:],
                                 inT_sb[:, g * 512:(g + 1) * 512],
                                 start=False, stop=True)
                nc.scalar.activation(outT_sb[:, g * 512:(g + 1) * 512],
                                     hp[:], AF.Relu, bias=bb[:])
            # node-major store + allgather for next layer's gather source
            if h_loc is not None:
                for b in range(NBLK):
                    tp = psT.tile([128, 128], BF16, tag="tp",
                                  name=f"tp{li}_{b}")
                    nc.tensor.transpose(
                        tp[:], outT_sb[:, b * 128:(b + 1) * 128],
                        ident_sb[:])
                    nm = nm_pool.tile([128, 128], BF16, tag="nm",
                                      name=f"nm{li}_{b}")
                    nc.scalar.copy(nm[:], tp[:])
                    nc.sync.dma_start(h_loc[b * 128:(b + 1) * 128, :],
                                      nm[:])
                nc.gpsimd.collective_compute(
                    "AllGather", ALU.bypass,
                    replica_groups=[list(range(NCORES))],
                    ins=[h_loc.opt()], outs=[h_full.opt()])

        gcn_layer(0, x_full_in, xT_sb, h1T_sb, h1_loc, h1_full)
        gcn_layer(1, h1_full, h1T_sb, h2T_sb, h2_loc, h2_full)
        gcn_layer(2, h2_full, h2T_sb, h3T_sb, None, None)

        # ---- pooling: pooledT[h, g] = sum_n h3[n, h] * (batch[n] == g) ----
        pool_ps = psP.tile([H, G], F32)
        for b in range(NBLK):
            tp = psT.tile([128, 128], BF16, tag="tp", name=f"tpp_{b}")
            nc.tensor.transpose(tp[:], h3T_sb[:, b * 128:(b + 1) * 128],
                                ident_sb[:])
            nm = nm_pool.tile([128, 128], BF16, tag="nm", name=f"nmp_{b}")
            nc.scalar.copy(nm[:], tp[:])
            pb = m_pool.tile([128, 128], BF16, tag="m", name=f"pb_{b}")
            nc.vector.tensor_scalar(pb[:], iota_sb[:],
                                    batchrel_sb[:, b:b + 1], None,
                                    ALU.is_equal)
            nc.tensor.matmul(pool_ps[:], nm[:], pb[:],
                             start=(b == 0), stop=(b == NBLK - 1))
        poolT_sb = sm_pool.tile([H, G], F32)
        nc.vector.tensor_copy(poolT_sb[:], pool_ps[:])
        nc.sync.dma_start(pool_in[:], poolT_sb[:])
        nc.gpsimd.collective_compute(
            "AllReduce", ALU.add, replica_groups=[list(range(NCORES))],
            ins=[pool_in.opt()], outs=[pool_out.opt()])
        poolT_full = sm_pool.tile([H, G], F32)
        nc.sync.dma_start(poolT_full[:], pool_out[:])

        # ---- logits = pooled @ w_out + b_out, then log_softmax ----
        log_ps = psB.tile([H, 512], F32, tag="hp", name="log_ps")
        nc.tensor.matmul(log_ps[:G, :C_OUT], poolT_full[:], wout_sb[:],
                         start=True, stop=False)
        nc.tensor.matmul(log_ps[:G, :C_OUT], ones_sb[:], bout_sb[:],
                         start=False, stop=True)
        logits = sm_pool.tile([G, C_OUT], F32)
        nc.vector.tensor_copy(logits[:], log_ps[:G, :C_OUT])
        mx = sm_pool.tile([G, 1], F32)
        nc.vector.tensor_reduce(mx[:], logits[:], mybir.AxisListType.X,
                                ALU.max)
        negmx = sm_pool.tile([G, 1], F32)
        nc.scalar.mul(negmx[:], mx[:], -1.0)
        expv = sm_pool.tile([G, C_OUT], F32)
        nc.scalar.activation(expv[:], logits[:], AF.Exp, bias=negmx[:])
        sm = sm_pool.tile([G, 1], F32)
        nc.vector.tensor_reduce(sm[:], expv[:], mybir.AxisListType.X,
                                ALU.add)
        lse = sm_pool.tile([G, 1], F32)
        nc.scalar.activation(lse[:], sm[:], AF.Ln)
        mxlse = sm_pool.tile([G, 1], F32)
        nc.vector.tensor_add(mxlse[:], mx[:], lse[:])
        outv = sm_pool.tile([G, C_OUT], F32)
        nc.vector.tensor_scalar(outv[:], logits[:], mxlse[:], None,
                                ALU.subtract)
        nc.sync.dma_start(out_t[:], outv[:])

    nc.compile()
    return nc


_CACHE = {}


def _weights_map(w1_rel, b1, w1_root, w2_rel, b2, w2_root, w3_rel, b3,
                 w3_root, w_out, b_out):
    def pad128(w):
        w = np.asarray(w, np.float32)
        if w.shape[0] < H:
            w = np.concatenate(
                [w, np.zeros((H - w.shape[0], w.shape[1]), np.float32)],
                axis=0)
        return w.astype(ml_dtypes.bfloat16)

    return {
        "w1_rel": pad128(w1_rel),
        "w1_root": pad128(w1_root),
        "w2_rel": pad128(w2_rel),
        "w2_root": pad128(w2_root),
        "w3_rel": pad128(w3_rel),
        "w3_root": pad128(w3_root),
        "b1": np.asarray(b1, np.float32).reshape(H, 1),
        "b2": np.asarray(b2, np.float32).reshape(H, 1),
        "b3": np.asarray(b3, np.float32).reshape(H, 1),
        "w_out": np.asarray(w_out, np.float32),
        "b_out": np.asarray(b_out, np.float32).reshape(1, C_OUT),
    }


def kernel(x, edge_index, batch, w1_rel, b1, w1_root, w2_rel, b2, w2_root,
           w3_rel, b3, w3_root, w_out, b_out):
    in_maps, cchunks = _prep_inputs(x, edge_index, batch)
    weights = _weights_map(w1_rel, b1, w1_root, w2_rel, b2, w2_root,
                           w3_rel, b3, w3_root, w_out, b_out)
    for m in in_maps:
        m.update(weights)

    if cchunks not in _CACHE:
        _CACHE[cchunks] = _build_program(cchunks)
    nc = _CACHE[cchunks]
    res = run_bass_kernel_spmd(nc, in_maps, core_ids=list(range(NCORES)))
    return np.asarray(res.results[0]["out"], np.float32)
